# revision 16
# baseline (speedup 1.0000x reference)
"""BiLSTM-CRF Trainium2 kernel (8 NeuronCores, data-parallel over batch).

B=128,T=256 -> char emb(200)+intent emb(56) -> 2-layer BiLSTM(H=512)
-> tag projection (K=62) -> CRF loss (logZ - gold) + viterbi decode.

Sharding: batch 128 -> 16 per core; params replicated; T sequential per core.
Device: input GEMMs, 4 LSTM recurrences, feats, CRF-forward logZ, viterbi
forward maxes. Host: embedding gather, gold gathers, viterbi backtrace.
"""

import sys

import numpy as np

sys.path.insert(0, "/opt/trn_rl_repo")

B, T, V_CH, V_IN, CH_D, IN_D, H, L = 128, 256, 10000, 50, 200, 56, 512, 60
K = L + 2
START, STOP = L, L + 1
NCORES = 8
NB = B // NCORES          # 16 samples per core
G4 = 4 * H                # 2048
NEG = -1e30

_cache = {}


def _gate_perm():
    # reorder [*, 4H] gate columns: chunk k (1024) = [i,i,f,f,g,g,o,o] for
    # h-slices 2k,2k+1 (128 each) -> contiguous activation spans per chunk
    idx = []
    for k in range(2):
        for q in range(4):
            for cc in range(2):
                idx.append(q * H + (2 * k + cc) * 128 + np.arange(128))
    return np.concatenate(idx)


def _build_consts(inputs):
    f32 = np.float32
    perm = _gate_perm()
    trans = np.asarray(inputs["trans"], f32)
    wih0 = np.asarray(inputs["wih0"], f32)
    whh0 = np.asarray(inputs["whh0"], f32)
    b0 = np.asarray(inputs["b0"], f32)
    wih1 = np.asarray(inputs["wih1"], f32)
    whh1 = np.asarray(inputs["whh1"], f32)
    b1 = np.asarray(inputs["b1"], f32)
    w_tag = np.asarray(inputs["w_tag"], f32)
    b_tag = np.asarray(inputs["b_tag"], f32)

    c = {}
    w0t = np.zeros((128, 2, 3, G4), f32)
    for d in range(2):
        wt = wih0[d].T[:, perm]
        w0t[:, d, 0] = wt[:128]
        w0t[:, d, 1] = wt[128:256]
        w0t[0, d, 2] = b0[d][perm]
    c["w0t"] = w0t
    whh0t = np.zeros((128, 2, 4, G4), f32)
    whh1t = np.zeros((128, 2, 4, G4), f32)
    for d in range(2):
        for ct in range(4):
            whh0t[:, d, ct] = whh0[d].T[ct * 128:(ct + 1) * 128][:, perm]
            whh1t[:, d, ct] = whh1[d].T[ct * 128:(ct + 1) * 128][:, perm]
    c["whh0t"] = whh0t
    c["whh1t"] = whh1t
    wih1t = np.zeros((128, 2, 9, G4), f32)
    for d in range(2):
        wt = wih1[d].T[:, perm]
        for ct in range(8):
            wih1t[:, d, ct] = wt[ct * 128:(ct + 1) * 128]
        wih1t[0, d, 8] = b1[d][perm]
    c["wih1t"] = wih1t
    wtagt = np.zeros((128, 2, 4, K), f32)
    for d in range(2):
        for s in range(4):
            wtagt[:, d, s] = w_tag.T[d * 512 + s * 128: d * 512 + (s + 1) * 128]
    c["wtagt"] = wtagt
    c["btagr"] = b_tag[None, :].astype(f32)                     # [1, 62]
    c["trstart"] = np.tile(trans[START][None, :], (NB, 1))      # [16, 62]
    c["trstopb"] = np.tile(trans[:, STOP][None, :], (NB, 1))
    c["expt"] = np.exp(trans).astype(f32)
    transpp = np.full((128, 8, K), NEG, f32)
    for p in range(128):
        jb = p % 8
        for ji in range(8):
            j = jb * 8 + ji
            if j < K:
                transpp[p, ji] = trans[:, j]
    c["transpp"] = transpp
    blockmask = np.zeros((128, 8, 8), f32)
    for p in range(128):
        blockmask[p, p % 8] = 1.0
    c["blockmask"] = blockmask
    rep16 = np.zeros((NB, 128), f32)
    for p in range(128):
        rep16[p // 8, p] = 1.0
    c["rep16"] = rep16
    c["col16"] = np.ascontiguousarray(rep16.T)                  # [128, 16]
    c["i16"] = np.eye(NB, dtype=f32)
    c["ones1x128"] = np.ones((1, 128), f32)
    c["tiny16"] = np.full((NB, 1), 1e-30, f32)
    return c


def _build_bass(t_steps):
    import concourse.bass as bass
    import concourse.mybir as mybir
    import concourse.tile as tile
    from concourse import bacc

    f32 = mybir.dt.float32
    ALU = mybir.AluOpType
    ACTF = mybir.ActivationFunctionType
    AX = mybir.AxisListType
    TT = t_steps
    TBL = TT * NB
    NBLK = TBL // 128            # (t,b) blocks of 128 rows (8 steps)
    HTW = min(16, TT)            # steps per hT DRAM block
    NTB = TT // HTW

    nc = bacc.Bacc("TRN2", target_bir_lowering=False, debug=False)

    P = {}
    for nm, shp in [
        ("x0t", [128, 3, TBL]), ("w0t", [128, 2, 3, G4]),
        ("whh0t", [128, 2, 4, G4]), ("wih1t", [128, 2, 9, G4]),
        ("whh1t", [128, 2, 4, G4]), ("wtagt", [128, 2, 4, K]),
        ("btagr", [1, K]), ("trstart", [NB, K]), ("trstopb", [NB, K]),
        ("expt", [K, K]), ("transpp", [128, 8, K]),
        ("blockmask", [128, 8, 8]), ("rep16", [NB, 128]),
        ("col16", [128, NB]), ("i16", [NB, NB]), ("ones1x128", [1, 128]),
        ("tiny16", [NB, 1]),
    ]:
        P[nm] = nc.declare_dram_parameter(nm, shp, f32, isOutput=False)
    P["mask01"] = nc.declare_dram_parameter("mask01", [NB, TT],
                                            mybir.dt.int32, isOutput=False)

    feats_o = nc.declare_dram_parameter("feats", [NB, TT, K], f32, isOutput=True)
    ahist_o = nc.declare_dram_parameter("ahist", [NB, TT, K], f32, isOutput=True)
    logz_o = nc.declare_dram_parameter("logz", [NB, 1], f32, isOutput=True)

    g0d = nc.dram_tensor("g0d", [2, NBLK, 128, G4], f32)
    g1d = nc.dram_tensor("g1d", [2, NBLK, 128, G4], f32)
    h0td = nc.dram_tensor("h0td", [2, 4, NTB, 128, HTW * NB], f32)

    with tile.TileContext(nc) as tc:
        with tc.tile_pool(name="const", bufs=1) as cp, \
             tc.tile_pool(name="pers", bufs=1) as pp:
            sb = {}
            for nm in ["wtagt", "btagr", "trstart", "trstopb", "expt",
                       "transpp", "blockmask", "rep16", "col16", "i16",
                       "ones1x128", "tiny16", "mask01"]:
                tl = cp.tile(list(P[nm].shape), P[nm].dtype, name=f"sb_{nm}")
                nc.sync.dma_start(out=tl[:], in_=P[nm][:])
                sb[nm] = tl

            feats = pp.tile([NB, TT, K], f32, name="featsb")
            cbuf = [pp.tile([NB, H], f32, name=f"c{d}") for d in range(2)]
            hbm = [pp.tile([NB, H], f32, name=f"h{d}") for d in range(2)]
            hTd = [pp.tile([128, 4, NB], f32, name=f"hT{d}") for d in range(2)]

            # ---------------- phase B: L0 input GEMM ----------------
            with tc.tile_pool(name="bs", bufs=1) as bsp, \
                 tc.tile_pool(name="bp", bufs=1, space="PSUM") as bpp:
                x0t = bsp.tile([128, 3, TBL], f32, name="sb_x0t")
                nc.sync.dma_start(out=x0t[:], in_=P["x0t"][:])
                w0t = bsp.tile([128, 2, 3, G4], f32, name="sb_w0t")
                nc.sync.dma_start(out=w0t[:], in_=P["w0t"][:])
                for d in range(2):
                    for blk in range(NBLK):
                        bsl = slice(blk * 128, (blk + 1) * 128)
                        ps = bpp.tile([128, G4], f32, tag="gps", bufs=2)
                        for ch in range(4):
                            cs = slice(ch * 512, (ch + 1) * 512)
                            nc.tensor.matmul(ps[:, cs], x0t[:, 0, bsl],
                                             w0t[:, d, 0, cs],
                                             start=True, stop=False)
                            nc.tensor.matmul(ps[:, cs], x0t[:, 1, bsl],
                                             w0t[:, d, 1, cs],
                                             start=False, stop=False)
                            nc.tensor.matmul(ps[:, cs], x0t[0:1, 2, bsl],
                                             w0t[0:1, d, 2, cs],
                                             start=False, stop=True)
                        gsb = bsp.tile([128, G4], f32, tag="gsb", bufs=3)
                        nc.vector.tensor_copy(gsb[:], ps[:])
                        nc.sync.dma_start(out=g0d[d, blk], in_=gsb[:])

            # ------------- LSTM recurrence pass (shared) -------------
            def lstm_pass(dirs, gsrc, whht, store_h0, feats_mode):
                # feats_mode: 0 none, 1 init (bwd), 2 accumulate (fwd)
                with tc.tile_pool(name="rs", bufs=1) as srp, \
                     tc.tile_pool(name="rp", bufs=1, space="PSUM") as prp:
                    wh = srp.tile([128, len(dirs), 4, G4], f32,
                                  name="wh" + "".join(map(str, dirs)))
                    for di, d in enumerate(dirs):
                        nc.sync.dma_start(out=wh[:, di], in_=whht[:, d])
                    hacc = {}
                    if store_h0:
                        for d in dirs:
                            hacc[d] = srp.tile([128, 4, HTW, NB], f32,
                                               tag=f"hacc{d}", bufs=2,
                                               name=f"haccN{d}")
                    for tt in range(TT):
                        for di, d in enumerate(dirs):
                            t = tt if d == 0 else TT - 1 - tt
                            first = (tt == 0)
                            gsl = srp.tile([NB, G4], f32, tag="gsl", bufs=2)
                            nc.sync.dma_start(
                                out=gsl[:],
                                in_=gsrc[d, t // 8, (t % 8) * NB:(t % 8) * NB + NB, :])
                            for kk in range(2):
                                gp = prp.tile([NB, 1024], f32, tag="gp", bufs=2)
                                for sub in range(2):
                                    cs = slice(sub * 512, (sub + 1) * 512)
                                    gcs = slice(kk * 1024 + sub * 512,
                                                kk * 1024 + (sub + 1) * 512)
                                    nc.tensor.matmul(gp[:, cs], sb["i16"][:],
                                                     gsl[:, gcs],
                                                     start=True, stop=first)
                                    if not first:
                                        for ct in range(4):
                                            nc.tensor.matmul(
                                                gp[:, cs], hTd[d][:, ct, :],
                                                wh[:, di, ct, gcs],
                                                start=False, stop=(ct == 3))
                                act = srp.tile([NB, 1024], f32, tag="act", bufs=4)
                                nc.scalar.activation(act[:, 0:512],
                                                     gp[:, 0:512], ACTF.Sigmoid)
                                nc.scalar.activation(act[:, 512:768],
                                                     gp[:, 512:768], ACTF.Tanh)
                                nc.scalar.activation(act[:, 768:1024],
                                                     gp[:, 768:1024], ACTF.Sigmoid)
                                cc = slice(kk * 256, (kk + 1) * 256)
                                if first:
                                    nc.vector.tensor_tensor(
                                        cbuf[d][:, cc], act[:, 0:256],
                                        act[:, 512:768], op=ALU.mult)
                                else:
                                    tmp = srp.tile([NB, 256], f32, tag="tmp",
                                                   bufs=4)
                                    nc.vector.tensor_tensor(
                                        tmp[:], act[:, 0:256], act[:, 512:768],
                                        op=ALU.mult)
                                    nc.vector.tensor_tensor(
                                        cbuf[d][:, cc], act[:, 256:512],
                                        cbuf[d][:, cc], op=ALU.mult)
                                    nc.vector.tensor_tensor(
                                        cbuf[d][:, cc], cbuf[d][:, cc],
                                        tmp[:], op=ALU.add)
                                thc = srp.tile([NB, 256], f32, tag="thc", bufs=4)
                                nc.scalar.activation(thc[:], cbuf[d][:, cc],
                                                     ACTF.Tanh)
                                nc.vector.tensor_tensor(
                                    hbm[d][:, cc], act[:, 768:1024], thc[:],
                                    op=ALU.mult)
                            ptr = prp.tile([128, 4, NB], f32, tag="ptr", bufs=2)
                            for s in range(4):
                                nc.tensor.transpose(
                                    ptr[:, s, :],
                                    hbm[d][:, s * 128:(s + 1) * 128],
                                    sb["i16"][:])
                            nc.scalar.copy(hTd[d][:], ptr[:])
                            if store_h0:
                                nc.vector.tensor_copy(
                                    hacc[d][:, :, t % HTW, :], ptr[:])
                                done = (t % HTW == HTW - 1) if d == 0 \
                                    else (t % HTW == 0)
                                if done:
                                    for ct in range(4):
                                        nc.sync.dma_start(
                                            out=h0td[d, ct, t // HTW],
                                            in_=hacc[d][:, ct, :, :])
                                    hacc[d] = srp.tile([128, 4, HTW, NB], f32,
                                                       tag=f"hacc{d}", bufs=2,
                                                       name=f"haccR{d}")
                            if feats_mode:
                                pf = prp.tile([NB, K], f32, tag="pf", bufs=2)
                                if feats_mode == 1:
                                    nc.tensor.matmul(
                                        pf[:], sb["ones1x128"][:, 0:NB],
                                        sb["btagr"][:], start=True, stop=False)
                                for s in range(4):
                                    nc.tensor.matmul(
                                        pf[:], hTd[d][:, s, :],
                                        sb["wtagt"][:, d, s],
                                        start=(feats_mode == 2 and s == 0),
                                        stop=(s == 3))
                                if feats_mode == 1:
                                    nc.vector.tensor_copy(feats[:, t, :], pf[:])
                                else:
                                    nc.vector.tensor_tensor(
                                        feats[:, t, :], feats[:, t, :], pf[:],
                                        op=ALU.add)

            # ---------------- phase C: L0 recurrence ----------------
            lstm_pass([0, 1], g0d, P["whh0t"], store_h0=True, feats_mode=0)

            # ---------------- phase D: L1 input GEMM ----------------
            with tc.tile_pool(name="ds", bufs=1) as dsp, \
                 tc.tile_pool(name="dp", bufs=1, space="PSUM") as dpp:
                for d in range(2):
                    w1 = dsp.tile([128, 9, G4], f32, tag="w1sb", bufs=1,
                                  name=f"w1sb{d}")
                    nc.sync.dma_start(out=w1[:], in_=P["wih1t"][:, d])
                    for blk in range(NBLK):
                        tb, sub = blk // (HTW // 8), blk % (HTW // 8)
                        cs0 = slice(sub * 128, (sub + 1) * 128)
                        lh = dsp.tile([128, 8, 128], f32, tag="lh", bufs=3)
                        for dd in range(2):
                            for ct in range(4):
                                nc.sync.dma_start(
                                    out=lh[:, dd * 4 + ct, :],
                                    in_=h0td[dd, ct, tb][:, cs0])
                        ps = dpp.tile([128, G4], f32, tag="g1ps", bufs=2)
                        for ch in range(4):
                            cs = slice(ch * 512, (ch + 1) * 512)
                            for ct in range(8):
                                nc.tensor.matmul(ps[:, cs], lh[:, ct, :],
                                                 w1[:, ct, cs],
                                                 start=(ct == 0), stop=False)
                            nc.tensor.matmul(ps[:, cs], sb["ones1x128"][:],
                                             w1[0:1, 8, cs],
                                             start=False, stop=True)
                        gsb = dsp.tile([128, G4], f32, tag="gsb", bufs=3)
                        nc.vector.tensor_copy(gsb[:], ps[:])
                        nc.sync.dma_start(out=g1d[d, blk], in_=gsb[:])

            # ------------- phase E: L1 recurrence (bwd, then fwd) -------------
            lstm_pass([1], g1d, P["whh1t"], store_h0=False, feats_mode=1)
            lstm_pass([0], g1d, P["whh1t"], store_h0=False, feats_mode=2)

            # ---------------- phase F: CRF logZ + viterbi ----------------
            with tc.tile_pool(name="fs", bufs=1) as scp, \
                 tc.tile_pool(name="fp", bufs=1, space="PSUM") as pcp:
                lza = scp.tile([NB, K], f32, name="lzalpha")
                nc.vector.tensor_tensor(lza[:], sb["trstart"][:],
                                        feats[:, 0, :], op=ALU.add)
                avr = scp.tile([NB, 8, K], f32, tag="avr", bufs=2)
                nc.vector.tensor_tensor(avr[:, 0, :], sb["trstart"][:],
                                        feats[:, 0, :], op=ALU.add)
                av_prev = avr[:, 0, :]
                for t in range(1, TT):
                    mk = sb["mask01"][:, t:t + 1]
                    mkb = bass.broadcast_tensor_aps(mk, lza[:])[0]
                    # --- logZ ---
                    nm = scp.tile([NB, 1], f32, tag="nm", bufs=2)
                    nc.vector.tensor_reduce(nm[:], lza[:], axis=AX.X,
                                            op=ALU.max, negate=True)
                    ea = scp.tile([NB, K], f32, tag="ea", bufs=2)
                    nc.scalar.activation(ea[:], lza[:], ACTF.Exp, bias=nm[:])
                    eaT = pcp.tile([K, NB], f32, tag="eaT", bufs=2)
                    nc.tensor.transpose(eaT[:], ea[:], sb["i16"][:])
                    eaTs = scp.tile([K, NB], f32, tag="eaTs", bufs=2)
                    nc.vector.tensor_copy(eaTs[:], eaT[:])
                    zz = pcp.tile([NB, K], f32, tag="zz", bufs=2)
                    nc.tensor.matmul(zz[:], eaTs[:], sb["expt"][:],
                                     start=True, stop=True)
                    lzt = scp.tile([NB, K], f32, tag="lzt", bufs=2)
                    nc.scalar.activation(lzt[:], zz[:], ACTF.Ln,
                                         bias=sb["tiny16"][:])
                    cand = scp.tile([NB, K], f32, tag="cand", bufs=2)
                    nc.vector.scalar_tensor_tensor(
                        cand[:], lzt[:], nm[:], feats[:, t, :],
                        op0=ALU.subtract, op1=ALU.add)
                    nc.vector.copy_predicated(lza[:], mkb, cand[:])
                    # --- viterbi maxes ---
                    ax8 = pcp.tile([128, 1, K], f32, tag="ax8", bufs=2)
                    nc.tensor.matmul(ax8[:, 0, :], sb["rep16"][:], av_prev,
                                     start=True, stop=True)
                    sc = scp.tile([128, 8, K], f32, tag="sc", bufs=2)
                    a_b, t_b = bass.broadcast_tensor_aps(ax8[:],
                                                         sb["transpp"][:])
                    nc.vector.tensor_tensor(sc[:], a_b, t_b, op=ALU.add)
                    mxv = scp.tile([128, 1, 8], f32, tag="mxv", bufs=2)
                    nc.vector.tensor_reduce(mxv[:, 0, :], sc[:], axis=AX.X,
                                            op=ALU.max)
                    mvsp = scp.tile([128, 8, 8], f32, tag="mvsp", bufs=2)
                    m_b, b_b = bass.broadcast_tensor_aps(mxv[:],
                                                         sb["blockmask"][:])
                    nc.vector.tensor_tensor(mvsp[:], m_b, b_b, op=ALU.mult)
                    a2 = pcp.tile([NB, 64], f32, tag="a2", bufs=2)
                    nc.tensor.matmul(a2[:], sb["col16"][:], mvsp[:],
                                     start=True, stop=True)
                    candv = scp.tile([NB, K], f32, tag="candv", bufs=2)
                    nc.vector.tensor_tensor(candv[:], a2[:, 0:K],
                                            feats[:, t, :], op=ALU.add)
                    av_new = avr[:, t % 8, :]
                    nc.vector.tensor_copy(av_new, av_prev)
                    nc.vector.copy_predicated(av_new, mkb, candv[:])
                    av_prev = av_new
                    if t % 8 == 7 or t == TT - 1:
                        t0 = (t // 8) * 8
                        nc.sync.dma_start(
                            out=ahist_o[:, t0:t + 1, :],
                            in_=avr[:, t0 % 8:t % 8 + 1, :])
                        if t < TT - 1:
                            avr = scp.tile([NB, 8, K], f32, tag="avr", bufs=2)
                            nc.vector.tensor_copy(avr[:, 7, :], av_prev)
                            av_prev = avr[:, 7, :]
                # ahist_o[:, 0, :] = init alpha: stored via first ring DMA
                # (t0=0 block includes slot 0)  -- handled above
                # --- final logZ ---
                fz = scp.tile([NB, K], f32, name="fz")
                nc.vector.tensor_tensor(fz[:], lza[:], sb["trstopb"][:],
                                        op=ALU.add)
                nm2 = scp.tile([NB, 1], f32, name="nm2")
                nc.vector.tensor_reduce(nm2[:], fz[:], axis=AX.X,
                                        op=ALU.max, negate=True)
                ez = scp.tile([NB, K], f32, name="ez")
                s2 = scp.tile([NB, 1], f32, name="s2")
                nc.scalar.activation(ez[:], fz[:], ACTF.Exp, bias=nm2[:],
                                     accum_out=s2[:])
                lg2 = scp.tile([NB, 1], f32, name="lg2")
                nc.scalar.activation(lg2[:], s2[:], ACTF.Ln,
                                     bias=sb["tiny16"][:])
                lgz = scp.tile([NB, 1], f32, name="lgz")
                nc.vector.tensor_scalar(lgz[:], lg2[:], nm2[:], None,
                                        op0=ALU.subtract)
                nc.sync.dma_start(out=logz_o[:], in_=lgz[:])
                nc.sync.dma_start(out=feats_o[:], in_=feats[:])

    nc.compile()
    return nc


def _run_device(inputs, t_steps):
    from concourse.bass_utils import run_bass_kernel_spmd

    if t_steps not in _cache:
        _cache[t_steps] = _build_bass(t_steps)
    nc = _cache[t_steps]

    f32 = np.float32
    consts = _build_consts(inputs)
    char_emb = np.asarray(inputs["char_emb"], f32)
    intent_emb = np.asarray(inputs["intent_emb"], f32)
    bc = np.asarray(inputs["batch_char"])
    bi = np.asarray(inputs["batch_intent"])
    mask = np.asarray(inputs["mask"]).astype(bool)[:, :t_steps]

    ce = char_emb[bc[:, :t_steps]]
    ie = intent_emb[bi[:, 0]]
    x0 = np.concatenate(
        [ce, np.broadcast_to(ie[:, None, :], (B, t_steps, IN_D))], axis=2)

    in_maps = []
    for c in range(NCORES):
        s = slice(c * NB, (c + 1) * NB)
        TBL = t_steps * NB
        xt = x0[s].transpose(2, 1, 0).reshape(256, TBL)
        x0t = np.zeros((128, 3, TBL), f32)
        x0t[:, 0] = xt[:128]
        x0t[:, 1] = xt[128:256]
        x0t[0, 2] = 1.0
        m = dict(consts)
        m["x0t"] = x0t
        m["mask01"] = np.ascontiguousarray(mask[s]).astype(np.int32)
        in_maps.append(m)

    res = run_bass_kernel_spmd(nc, in_maps, list(range(NCORES)))
    return res.results


def kernel(batch_char, batch_intent, batch_char_len, mask, batch_label,
           char_emb, intent_emb, wih0, whh0, b0, wih1, whh1, b1,
           w_tag, b_tag, trans, _t_steps=T):
    f32 = np.float32
    TT = _t_steps
    inputs = dict(batch_char=batch_char, batch_intent=batch_intent,
                  batch_char_len=batch_char_len, mask=mask,
                  batch_label=batch_label, char_emb=char_emb,
                  intent_emb=intent_emb, wih0=wih0, whh0=whh0, b0=b0,
                  wih1=wih1, whh1=whh1, b1=b1, w_tag=w_tag, b_tag=b_tag,
                  trans=trans)
    results = _run_device(inputs, TT)

    trans = np.asarray(trans, f32)
    mask = np.asarray(mask).astype(bool)[:, :TT]
    labels = np.asarray(batch_label)[:, :TT]

    feats = np.concatenate([np.asarray(r["feats"]) for r in results], 0)
    ahist = np.concatenate([np.asarray(r["ahist"]) for r in results], 0)
    logz = np.concatenate([np.asarray(r["logz"]) for r in results], 0)[:, 0]

    m = mask.astype(f32)
    emit = np.take_along_axis(feats, labels[..., None], axis=2)[..., 0] * m
    tr = trans[labels[:, :-1], labels[:, 1:]] * m[:, 1:]
    start = trans[START, labels[:, 0]]
    last_idx = mask.sum(1).astype(np.int64) - 1
    last_tag_g = labels[np.arange(B), last_idx]
    gold = emit.sum(1) + tr.sum(1) + start + trans[last_tag_g, STOP]
    loss = np.float32(np.sum(logz.astype(np.float64) - gold))

    final = ahist[:, TT - 1] + trans[:, STOP][None, :]
    tag = np.argmax(final, axis=1).astype(np.int32)
    path = np.zeros((B, TT), np.int32)
    path[:, TT - 1] = tag
    for t in range(TT - 2, -1, -1):
        sc = ahist[:, t] + trans[:, tag].T
        nt = np.argmax(sc, axis=1).astype(np.int32)
        tag = np.where(mask[:, t + 1], nt, tag)
        path[:, t] = tag
    path = path * mask.astype(np.int32)
    return loss, path
